# revision 1
# baseline (speedup 1.0000x reference)
"""5G LDPC BG1 encoder (k=8000, n=16000, r=0.5, Z=384) on 8 Trainium2 cores.

Strategy: pure data parallelism over the batch (2048 -> 8 cores x 256 rows,
2 partition-tiles of 128 each). Bits are kept as bf16 0.0/1.0 on the free
axis; GF(2) addition is bitwise XOR on the raw bit patterns (0x3F80 ^ 0x3F80
= 0x0000), so no mod-2 is ever needed. Circulant lifted blocks (Z=384) are
handled by keeping a duplicated "halo" copy of every 384-col block so a
cyclic shift is a single contiguous slice -> one elementwise op per
base-graph entry.  Rate matching only emits extension parity bits pb[0:7232]
(19 of 42 blocks), so the other 23 blocks are never computed.  The output
interleaver (out[:, 4j+i] = c_short[i*4000+j]) is fused with the bf16->f32
up-conversion as stride-4 copies on the Activation engine, emitted per
4000-column output chunk so chunk DMAs overlap compute.  XOR work is split
DVE/GpSimd to balance engine busy time.
"""
import numpy as np
from contextlib import ExitStack

Z = 384
KB = 22
MB = 46
K = 8000
N = 16000
K_LDPC = KB * Z          # 8448
M_A = 4 * Z              # 1536
NBPS = 4
NQ = N // NBPS           # 4000
PB_BLOCKS = 19           # only pb[0:7232] survives rate matching

B_TOTAL = 2048
N_CORES = 8
B_CORE = B_TOTAL // N_CORES   # 256
P = 128
TILES = B_CORE // P           # 2
NCHUNK = 4                    # output column chunks of 4000

_CACHE = {}


def _base_entries(rows, cols):
    """Recover (base_row, base_col, shift) triplets from lifted index lists."""
    rows = np.asarray(rows, np.int64)
    cols = np.asarray(cols, np.int64)
    m = (rows % Z) == 0
    br = (rows[m] // Z).astype(int)
    bc = (cols[m] // Z).astype(int)
    sh = (cols[m] % Z).astype(int)
    return list(zip(br.tolist(), bc.tolist(), sh.tolist()))


def _group(entries, n_blocks, drop_bc=()):
    g = [[] for _ in range(n_blocks)]
    for br, bc, s in entries:
        if bc in drop_bc or br >= n_blocks:
            continue
        g[br].append((bc, s))
    return g


def _ilv_copies(chunk):
    """Interleaver copy specs for output chunk (cols [chunk*4000, +4000)):
    (tile, blk0, off, nblk, ln, dst_start_within_chunk).

    c_short = u_bits[768:8000] ++ pa[0:1536] ++ pb[0:7232], and
    out[:, 4j+i] = c_short[i*4000 + j]; chunk c covers j in [c*1000,(c+1)*1000).
    """
    spans = ([("u", b, 0, Z) for b in range(2, 20)] + [("u", 20, 0, 320)]
             + [("pa", b, 0, Z) for b in range(4)]
             + [("pb", b, 0, Z) for b in range(18)] + [("pb", 18, 0, 320)])
    jlo, jhi = chunk * (NQ // NCHUNK), (chunk + 1) * (NQ // NCHUNK)
    out = []
    for i in range(NBPS):
        # phase i reads c_short[i*NQ + j] for j in [jlo, jhi) of this chunk
        glo, ghi = i * NQ + jlo, i * NQ + jhi
        g = 0
        pieces = []
        for tname, blk, off, ln in spans:
            a, b = max(g, glo), min(g + ln, ghi)
            if a < b:
                pieces.append((tname, blk, off + a - g, b - a,
                               4 * (a - glo) + i))
            g += ln
        merged = []
        for pc in pieces:
            tname, blk, off, ln, ds = pc
            if merged and off == 0 and ln == Z:
                mt, mb_, mo, mn, ml, mds = merged[-1]
                if mt == tname and mo == 0 and ml == Z and mb_ + mn == blk:
                    merged[-1] = (mt, mb_, mo, mn + 1, ml, mds)
                    continue
            merged.append((tname, blk, off, 1, ln, ds))
        out.extend(merged)
    return out


def _build_program(gA, gC1, gC2):
    import concourse.tile as tile
    from concourse import bacc, mybir
    from concourse.alu_op_type import AluOpType

    f32 = mybir.dt.float32
    u16 = mybir.dt.uint16
    bf16 = mybir.dt.bfloat16
    XOR = AluOpType.bitwise_xor

    nc = bacc.Bacc("TRN2", target_bir_lowering=False, debug=False)
    u_dram = nc.dram_tensor("u", [B_CORE, K], f32, kind="ExternalInput").ap()
    o_dram = nc.dram_tensor("out", [B_CORE, N], f32, kind="ExternalOutput").ap()

    with tile.TileContext(nc) as tc, ExitStack() as ctx:
        pin = ctx.enter_context(tc.tile_pool(name="pin", bufs=2))
        pw2 = ctx.enter_context(tc.tile_pool(name="pw2", bufs=2))
        pw1 = ctx.enter_context(tc.tile_pool(name="pw1", bufs=1))
        pout = ctx.enter_context(tc.tile_pool(name="pout", bufs=1))

        for t in range(TILES):
            r0 = t * P
            # ---- DMA in (block-aligned chunks) + convert to bf16 u_dup ----
            tf0 = pin.tile([P, 10, Z], f32, tag="tf")
            nc.sync.dma_start(tf0[:], u_dram[r0:r0 + P, 0:3840])
            tf1 = pin.tile([P, 10, Z], f32, tag="tf")
            nc.sync.dma_start(tf1[:], u_dram[r0:r0 + P, 3840:7680])
            tf2 = pin.tile([P, 320], f32, tag="tf2")
            nc.sync.dma_start(tf2[:], u_dram[r0:r0 + P, 7680:8000])

            # u_dup[p, bc, 0:384] = block bc ; [p, bc, 384:768] = same (halo)
            u_dup = pw2.tile([P, KB, 2 * Z], u16, tag="u_dup")
            nc.scalar.copy(u_dup[:, 0:10, 0:Z].bitcast(bf16), tf0[:])
            nc.scalar.copy(u_dup[:, 10:20, 0:Z].bitcast(bf16), tf1[:])
            nc.scalar.copy(u_dup[:, 20, 0:320].bitcast(bf16), tf2[:])
            nc.gpsimd.memset(u_dup[:, 20, 320:Z], 0)
            nc.gpsimd.memset(u_dup[:, 20, Z + 320:2 * Z], 0)
            nc.vector.tensor_copy(u_dup[:, 0:10, Z:2 * Z], u_dup[:, 0:10, 0:Z])
            nc.vector.tensor_copy(u_dup[:, 10:20, Z:2 * Z], u_dup[:, 10:20, 0:Z])
            nc.gpsimd.tensor_copy(u_dup[:, 20, Z:Z + 320], u_dup[:, 20, 0:320])

            def usrc(bc, s):
                return u_dup[:, bc, s:s + Z]

            def accumulate(eng, dst, srcs):
                """dst <- XOR of srcs (first pair direct, rest in place)."""
                if len(srcs) == 1:
                    nc.vector.tensor_copy(dst, srcs[0])
                    return
                eng.tensor_tensor(dst, srcs[0], srcs[1], op=XOR)
                for sp in srcs[2:]:
                    eng.tensor_tensor(dst, dst, sp, op=XOR)

            # ---- au = A @ u ----
            au = pw1.tile([P, 4, Z], u16, tag="au")
            for br in range(4):
                accumulate(nc.vector, au[:, br, :],
                           [usrc(bc, s) for bc, s in gA[br]])

            # ---- pa = B_inv @ au = cumulative XOR chain ----
            pa = pw1.tile([P, 4, 2 * Z], u16, tag="pa")
            nc.vector.tensor_copy(pa[:, 0, 0:Z], au[:, 0, :])
            for i in range(1, 4):
                nc.vector.tensor_tensor(pa[:, i, 0:Z], pa[:, i - 1, 0:Z],
                                        au[:, i, :], op=XOR)
            nc.gpsimd.tensor_copy(pa[:, :, Z:2 * Z], pa[:, :, 0:Z])

            def pasrc(bc, s):
                return pa[:, bc, s:s + Z]

            # ---- pb = C1 @ u + C2 @ pa (only the 19 surviving blocks) ----
            # Bitwise XOR is DVE-only on TRN2 (HW verifier rejects Pool).
            pb = pw1.tile([P, PB_BLOCKS, Z], u16, tag="pb")

            def pb_block(lr):
                srcs = [usrc(bc, s) for bc, s in gC1[lr]]
                srcs += [pasrc(bc, s) for bc, s in gC2[lr]]
                accumulate(nc.vector, pb[:, lr, :], srcs)

            # ---- interleave + bf16->f32 per output chunk, chunk DMA out ----
            # Early (u/pa-sourced, phases i=0,1) copies go to GpSimd so they
            # run during the DVE XOR burst; pb-sourced (i=2,3) go to ACT and
            # are emitted as soon as the pb blocks a chunk needs are done.
            tiles = {"u": u_dup, "pa": pa, "pb": pb}
            cw = N // NCHUNK

            def emit_ilv(of, c, want_pb):
                for tname, blk0, off, nblk, ln, ds in _ilv_copies(c):
                    if (tname == "pb") != want_pb:
                        continue
                    src_t = tiles[tname]
                    if nblk > 1:
                        dst = of[:, ds:ds + 4 * (Z * nblk - 1) + 1:4]
                        dst = dst.rearrange("p (a b) -> p a b", b=Z)
                        src = src_t[:, blk0:blk0 + nblk, 0:Z]
                    else:
                        dst = of[:, ds:ds + 4 * (ln - 1) + 1:4]
                        src = src_t[:, blk0, off:off + ln]
                    if want_pb:
                        nc.scalar.copy(dst, src.bitcast(bf16))
                    else:
                        nc.gpsimd.tensor_copy(dst, src.bitcast(bf16))

            # pb blocks needed per chunk (max block index + 1)
            need = []
            for c in range(NCHUNK):
                mx = 0
                for tname, blk0, off, nblk, ln, ds in _ilv_copies(c):
                    if tname == "pb":
                        mx = max(mx, blk0 + nblk)
                need.append(mx)

            done = 0
            for c in range(NCHUNK):
                of = pout.tile([P, cw], f32, tag=f"of{c % 2}")
                emit_ilv(of, c, want_pb=False)
                for lr in range(done, need[c]):
                    pb_block(lr)
                done = max(done, need[c])
                emit_ilv(of, c, want_pb=True)
                nc.sync.dma_start(o_dram[r0:r0 + P, c * cw:(c + 1) * cw],
                                  of[:])
            for lr in range(done, PB_BLOCKS):
                pb_block(lr)

    return nc


def _get_program(a_rows, a_cols, bi_rows, bi_cols, c1_rows, c1_cols,
                 c2_rows, c2_cols):
    if "prog" in _CACHE:
        return _CACHE["prog"]
    entB = _base_entries(bi_rows, bi_cols)
    assert sorted(entB) == [(i, j, 0) for i in range(4) for j in range(i + 1)]
    gA = _group(_base_entries(a_rows, a_cols), 4, drop_bc=(21,))
    gC1 = _group(_base_entries(c1_rows, c1_cols), PB_BLOCKS, drop_bc=(21,))
    gC2 = _group(_base_entries(c2_rows, c2_cols), PB_BLOCKS)
    nc = _build_program(gA, gC1, gC2)
    nc.compile()
    _CACHE["prog"] = nc
    return nc


def kernel(u, a_rows, a_cols, bi_rows, bi_cols, c1_rows, c1_cols,
           c2_rows, c2_cols, out_int, **_ignored):
    from concourse.bass_utils import run_bass_kernel_spmd

    u = np.ascontiguousarray(np.asarray(u, np.float32))
    assert u.shape == (B_TOTAL, K)
    oi = np.asarray(out_int)
    expect = np.arange(N, dtype=oi.dtype).reshape(NBPS, NQ).T.ravel()
    assert np.array_equal(oi, expect), "unexpected output interleaver"

    nc = _get_program(a_rows, a_cols, bi_rows, bi_cols,
                      c1_rows, c1_cols, c2_rows, c2_cols)
    in_maps = [{"u": u[i * B_CORE:(i + 1) * B_CORE]} for i in range(N_CORES)]
    res = run_bass_kernel_spmd(nc, in_maps, core_ids=list(range(N_CORES)))
    return np.concatenate([res.results[i]["out"] for i in range(N_CORES)], axis=0)

